# revision 5
# baseline (speedup 1.0000x reference)
"""Squared Euclidean distance matrix kernel for Trainium2 (Bass/Tile).

out[i, j] = ||mat_1[i]||^2 + ||mat_2[j]||^2 - 2 * mat_1[i] . mat_2[j]

Sharding: mat_1 rows (and hence output rows) split across 8 NeuronCores;
mat_2 replicated.  Each core computes a (2048, 8192) tile independently.

Per-core plan (v4 — fp8 DoubleRow GEMM + fp16 output):
  - Bt = fp8(-2 * B^T) in DoubleRow layout [128, 2, 8192] (PE transposes
    f32, DVE scale+cast).  At = fp8(A^T) likewise [128, 2, 2048].
  - Row vectors on partitions 0/1 drive a rank-2 correction matmul
    (f32r):  corr_lhsT = [ones_row; sqa_row], corr_rhs = [sqb_row;
    ones_row], so corr^T@corr = sq_a[m] + sq_b[n].  sq_* are computed
    from the QUANTIZED fp8 tiles (squares on DVE/GPSIMD, column-ones
    matmul reduce on PE) so the output is exactly ||a_q - b_q||^2 up to
    fp16 rounding.
  - main loop: one DoubleRow fp8 matmul (contracts all 256 dims, 0.5
    cyc/row) + the rank-2 correction matmul -> PSUM; copy to fp16
    staging (alternating DVE / ACT) -> 1MB contiguous DMA out (fp16).
  - Host upcasts the fp16 result to f32.
"""

import numpy as np

import concourse.bass as bass
import concourse.mybir as mybir
from concourse import bacc
from contextlib import ExitStack
from concourse.tile import TileContext
from concourse.masks import make_identity

F32 = mybir.dt.float32
F32R = mybir.dt.float32r
F16 = mybir.dt.float16
FP8 = mybir.dt.float8e4
AX = mybir.AxisListType
OP = mybir.AluOpType
AF = mybir.ActivationFunctionType
DR = mybir.MatmulPerfMode.DoubleRow

N_CORES = 8
M_FULL, N_FULL, D_FULL = 16384, 8192, 256


def build(m_sh=M_FULL // N_CORES, n=N_FULL, d=D_FULL):
    P = 128
    FD = 512                      # matmul moving free dim (1 PSUM bank fp32)
    KC = d // P                   # contraction chunks (2 -> DoubleRow depth)
    MT = m_sh // P                # m tiles per core
    NB = n // P                   # b row tiles
    AB = min(4, MT)               # a row tiles per load batch
    BB = min(4, NB)               # b row tiles per load batch
    out_w = min(4096, n)          # out staging width == half width
    OH = n // out_w               # number of halves
    SPW = out_w // FD             # 512-slices per half
    FDm = min(FD, m_sh)           # row-vector chunk width along m
    MS = m_sh // FDm

    nc = bacc.Bacc()
    a = nc.dram_tensor("a", [m_sh, d], F32, kind="ExternalInput")
    b = nc.dram_tensor("b", [n, d], F32, kind="ExternalInput")
    o = nc.dram_tensor("out", [m_sh, n], F16, kind="ExternalOutput")

    with ExitStack() as ctx:
        tc = ctx.enter_context(TileContext(nc))
        singles = ctx.enter_context(tc.tile_pool(name="singles", bufs=1))
        persist = ctx.enter_context(tc.tile_pool(name="persist", bufs=1))
        natp = ctx.enter_context(tc.tile_pool(name="natp", bufs=3))
        tmpp = ctx.enter_context(tc.tile_pool(name="tmpp", bufs=3))
        outp = ctx.enter_context(tc.tile_pool(name="outp", bufs=2))
        psump = ctx.enter_context(tc.tile_pool(name="psump", bufs=1, space="PSUM"))

        identity = singles.tile([P, P], F32, tag="identity", name="identity")
        make_identity(nc, identity)

        # f32r constants (memset f32 staging, rounded copy into f32r)
        ones_f = singles.tile([P, FD], F32, tag="ones_f", name="ones_f")
        nc.vector.memset(ones_f, 1.0)
        allones = singles.tile([P, FD], F32R, tag="allones", name="allones")
        nc.vector.tensor_copy(allones, ones_f)
        cst_f = singles.tile([P, 2], F32, tag="cst_f", name="cst_f")
        nc.vector.memset(cst_f[:, 0:1], 0.25)
        nc.vector.memset(cst_f[:, 1:2], 1.0 / 128.0)
        cst = singles.tile([P, 2], F32R, tag="cst", name="cst")
        nc.vector.tensor_copy(cst, cst_f)
        qcol = cst[:, 0:1]        # 0.25 column
        rcol = cst[:, 1:2]        # 1/128 column
        onecol = allones[:, 0:1]  # 1.0 column

        # fp8 DoubleRow operand tiles: [128, KC, width]
        bt8 = persist.tile([P, KC, n], FP8, tag="bt8", name="bt8")
        at8 = persist.tile([P, KC, m_sh], FP8, tag="at8", name="at8")
        corr_lhsT = persist.tile([2, m_sh], F32R, tag="corr_l", name="corr_lhsT")
        corr_rhs = persist.tile([2, n], F32R, tag="corr_r", name="corr_rhs")
        sqa_row = persist.tile([1, m_sh], F32R, tag="sqa_row", name="sqa_row")

        # ---- ones row: ps_ones = (1/128 col)^T @ allones = row of 1.0 ----
        ps_ones = psump.tile([1, FDm], F32, tag="row", bufs=1, name="ps_ones")
        nc.tensor.matmul(ps_ones, rcol, allones[:, :FDm], start=True, stop=True)
        for i in range(m_sh // FDm):
            nc.vector.tensor_copy(corr_lhsT[0:1, i * FDm:(i + 1) * FDm], ps_ones)

        # ---- A chain: load, transpose+cast to fp8, sq_a row, corr_lhsT ----
        for t4 in range(MT // AB):
            a_nat = natp.tile([P, AB, d], F32, tag="anat", bufs=2, name="a_nat")
            src = a[t4 * AB * P:(t4 + 1) * AB * P, :].rearrange(
                "(t p) d -> p t d", p=P
            )
            nc.scalar.dma_start(out=a_nat, in_=src)
            for j in range(AB):
                mt = t4 * AB + j
                for k in range(KC):
                    pt = psump.tile([P, P], F32, tag="tp", bufs=2, name="pt_a")
                    nc.tensor.transpose(pt, a_nat[:, j, k * P:(k + 1) * P], identity)
                    nc.vector.tensor_copy(at8[:, k, mt * P:(mt + 1) * P], pt)
        for s in range(MS):
            msl = slice(s * FDm, (s + 1) * FDm)
            ps = psump.tile([1, FDm], F32, tag="row", bufs=1, name="ps_sqa")
            for k in range(KC):
                asq = tmpp.tile([P, FDm], F32R, tag="bsq", bufs=2, name="asq")
                nc.gpsimd.tensor_mul(asq, at8[:, k, msl], at8[:, k, msl])
                nc.tensor.matmul(ps, onecol, asq, start=(k == 0), stop=(k == KC - 1))
            nc.vector.tensor_copy(sqa_row[0:1, msl], ps)
        nc.gpsimd.dma_start(out=corr_lhsT[1:2, :], in_=sqa_row[0:1, :])

        # ---- per half: B load/transpose + sqb per quad, then main loop ----
        bt_per_h = (NB // OH)     # b row tiles per half
        for h in range(OH):
            # ones row of corr_rhs for this half (needs only corr_lhsT[0])
            w = min(m_sh, out_w)
            for i in range(max(1, out_w // m_sh)):
                nc.gpsimd.dma_start(
                    out=corr_rhs[1:2, h * out_w + i * w: h * out_w + (i + 1) * w],
                    in_=corr_lhsT[0:1, :w],
                )
            # B load + transpose + scale+cast, one 512-wide quad at a time
            for q in range(bt_per_h // BB):
                t0 = h * bt_per_h + q * BB
                b_nat = natp.tile([P, BB, d], F32, tag="bnat", bufs=6,
                                  name="b_nat")
                src = b[t0 * P:(t0 + BB) * P, :].rearrange("(t p) d -> p t d", p=P)
                nc.scalar.dma_start(out=b_nat, in_=src)
                for j in range(BB):
                    t = t0 + j
                    for k in range(KC):
                        pt = psump.tile([P, P], F32, tag="tp", bufs=2, name="pt")
                        nc.tensor.transpose(
                            pt, b_nat[:, j, k * P:(k + 1) * P], identity
                        )
                        nc.vector.tensor_scalar_mul(
                            bt8[:, k, t * P:(t + 1) * P], pt, -2.0
                        )
            # sqb rows for this half: squares of fp8(-2b) on DVE, 0.25-col
            # matmul reduce on PE
            for sj in range(SPW):
                s = h * SPW + sj
                nsl = slice(s * FD, (s + 1) * FD)
                ps = psump.tile([1, FD], F32, tag="row", bufs=1, name="ps_sqb")
                for k in range(KC):
                    bsq = tmpp.tile([P, FD], F32R, tag="bsq", bufs=2,
                                    name="bsq")
                    nc.gpsimd.tensor_mul(bsq, bt8[:, k, nsl], bt8[:, k, nsl])
                    nc.tensor.matmul(
                        ps, qcol, bsq, start=(k == 0), stop=(k == KC - 1)
                    )
                nc.vector.tensor_copy(corr_rhs[0:1, nsl], ps)
            # main loop for this half
            for m in range(MT):
                msl = slice(m * P, (m + 1) * P)
                ostage = outp.tile([P, out_w], F16, tag="ostage", name="ostage")
                for sj in range(SPW):
                    s = h * SPW + sj
                    nsl = slice(s * FD, (s + 1) * FD)
                    ps = psump.tile([P, FD], F32, tag="mm", bufs=5, name="ps_mm")
                    nc.tensor.matmul(
                        ps, at8[:, :, msl], bt8[:, :, nsl],
                        start=True, stop=False, perf_mode=DR,
                    )
                    nc.tensor.matmul(
                        ps, corr_lhsT[:, msl], corr_rhs[:, nsl],
                        start=False, stop=True,
                    )
                    osl = ostage[:, sj * FD:(sj + 1) * FD]
                    # 60/40 ACT/DVE split (ACT is the faster copier and
                    # DVE carries the transpose-cast + staging work)
                    if (m * SPW + sj + h) % 5 < 2:
                        nc.vector.tensor_copy(osl, ps)
                    else:
                        nc.scalar.activation(osl, ps, AF.Copy)
                nc.sync.dma_start(
                    out=o[msl, h * out_w:(h + 1) * out_w], in_=ostage
                )
    nc.finalize()
    return nc


_CACHE = {}


def _get_nc():
    if "nc" not in _CACHE:
        _CACHE["nc"] = build()
    return _CACHE["nc"]


def run(mat_1, mat_2, trace=False, **kw):
    from concourse.bass_utils import run_bass_kernel_spmd

    a = np.ascontiguousarray(np.asarray(mat_1, dtype=np.float32))
    b = np.ascontiguousarray(np.asarray(mat_2, dtype=np.float32))
    assert a.shape == (M_FULL, D_FULL) and b.shape == (N_FULL, D_FULL)
    m_sh = M_FULL // N_CORES
    nc = _get_nc()
    in_maps = [
        {"a": a[c * m_sh:(c + 1) * m_sh], "b": b} for c in range(N_CORES)
    ]
    res = run_bass_kernel_spmd(
        nc, in_maps, core_ids=list(range(N_CORES)), trace=trace, **kw
    )
    out = np.concatenate(
        [np.asarray(r["out"], dtype=np.float32) for r in res.results], axis=0
    )
    return out, res


def kernel(mat_1, mat_2):
    return run(mat_1, mat_2)[0]


# revision 11
# speedup vs baseline: 1.6017x; 1.6017x over previous
"""Squared Euclidean distance matrix kernel for Trainium2 (Bass/Tile).

out[i, j] = ||mat_1[i]||^2 + ||mat_2[j]||^2 - 2 * mat_1[i] . mat_2[j]

Sharding: mat_1 rows (and output rows) split across 8 NeuronCores; mat_2
replicated.  Each core computes a (2048, 8192) tile independently.

v5 design (all-fp8-DoubleRow PE stream + fp16 output), based on HW
microbenchmarks:
  - at8 = fp8(A^T) [128, 2, 2048], bt8 = fp8(-2 B^T) [128, 2, 8192]
    (PE f32 transposes 4-at-a-time into one PSUM bank, 512-wide DVE
    scale-casts out).
  - Main tile (m, s): ONE DoubleRow fp8 matmul contracts all 256 dims
    (measured 216 ns), plus ONE zero-padded DoubleRow correction matmul
    (full K=128 partitions, only rows 0-1 nonzero; small-K matmuls and
    mixed-mode groups stall the PE stream, zero-padded same-shape DR
    does not):
      corr_l[0,c,m] = fp8(sq_a[m]/4) (hi, lo residual), corr_l[1,c,m]=4
      corr_r[0,c,n] = 4,  corr_r[1,c,n] = fp8(sq_b[n]/4) (hi, lo)
    so the pair contributes sq_a[m] + sq_b[n] exactly up to the fp8 lo
    residual (~0.5 abs).  sq_* are computed from the QUANTIZED tiles
    (squares on GPSIMD, fp8 0.25/1.0-column DoubleRow reduce on PE).
  - PSUM: two 1536-col (3-bank) main tiles double-buffered + 1 transpose
    bank + 1 row bank.  Evacuation in 1536/1024-wide chunks, split
    between DVE tensor_copy and ACT Copy, into fp16 staging; 1 MB DMA
    per (m, half).  Host upcasts fp16 -> f32.
"""

import numpy as np

import concourse.bass as bass
import concourse.mybir as mybir
from concourse import bacc
from contextlib import ExitStack
from concourse.tile import TileContext
from concourse.masks import make_identity

F32 = mybir.dt.float32
F32R = mybir.dt.float32r
F16 = mybir.dt.float16
FP8 = mybir.dt.float8e4
AX = mybir.AxisListType
OP = mybir.AluOpType
AF = mybir.ActivationFunctionType
DR = mybir.MatmulPerfMode.DoubleRow

N_CORES = 8
M_FULL, N_FULL, D_FULL = 16384, 8192, 256


def build(m_sh=M_FULL // N_CORES, n=N_FULL, d=D_FULL):
    P = 128
    FD = 512                      # psum bank width (f32)
    KC = d // P                   # 2 -> DoubleRow depth
    MT = m_sh // P                # m tiles per core (16)
    NB = n // P                   # b row tiles (64)
    AB = 4                        # row tiles per load batch
    out_w = 4096                  # out staging width == half width
    OH = n // out_w               # halves (2)
    SPH = out_w // FD             # 512-slices per half (8)
    # evacuation chunking per (m, half): 1536+1536+1024
    EV = [(0, 1536), (1536, 1536), (3072, 1024)]

    nc = bacc.Bacc()
    a = nc.dram_tensor("a", [m_sh, d], F32, kind="ExternalInput")
    b = nc.dram_tensor("b", [n, d], F32, kind="ExternalInput")
    o = nc.dram_tensor("out", [m_sh, n], F16, kind="ExternalOutput")

    with ExitStack() as ctx:
        tc = ctx.enter_context(TileContext(nc))
        singles = ctx.enter_context(tc.tile_pool(name="singles", bufs=1))
        persist = ctx.enter_context(tc.tile_pool(name="persist", bufs=1))
        natp = ctx.enter_context(tc.tile_pool(name="natp", bufs=3))
        outp = ctx.enter_context(tc.tile_pool(name="outp", bufs=2))
        psump = ctx.enter_context(tc.tile_pool(name="psump", bufs=1, space="PSUM"))

        identity = singles.tile([P, P], F32, tag="identity", name="identity")
        make_identity(nc, identity)

        # fp8 DR reduce columns [128, 2, 1]:
        #   A: sq_a/4 = 0.25 * sum(a^2)            -> 0.25
        #   B: sq_b/4 = (1/16) * sum((-2b)^2)      -> 0.0625
        # (DR weights need pair-stride %16 == 0 -> 16 columns wide, all
        # equal; the reduce result lands in psum rows 0..15, row 0 used)
        qcol8 = singles.tile([P, KC, 16], FP8, tag="qcol8", name="qcol8")
        nc.vector.memset(qcol8, 0.25)
        q16col8 = singles.tile([P, KC, 16], FP8, tag="q16col8", name="q16col8")
        nc.vector.memset(q16col8, 0.0625)

        # fp8 DR operand tiles
        bt8 = persist.tile([P, KC, n], FP8, tag="bt8", name="bt8")
        at8 = persist.tile([P, KC, m_sh], FP8, tag="at8", name="at8")
        # zero-padded DR correction operands
        c8l = persist.tile([P, KC, m_sh], FP8, tag="c8l", name="c8l")
        c8r = persist.tile([P, KC, n], FP8, tag="c8r", name="c8r")
        nc.gpsimd.memset(c8l, 0.0)
        nc.gpsimd.memset(c8r, 0.0)
        # compute engines cannot address partition offset 1; stage the 4.0
        # rows at partition 0 and DMA into place
        fours = singles.tile([1, KC, n], FP8, tag="fours", name="fours")
        nc.vector.memset(fours, 4.0)
        nc.gpsimd.dma_start(out=c8l[1:2, :, :], in_=fours[0:1, :, :m_sh])
        nc.gpsimd.dma_start(out=c8r[0:1, :, :], in_=fours[0:1, :, :])

        # row staging (partition 0), DMA'd into c8l/c8r partition rows
        rowst = persist.tile([1, 2, max(m_sh, n)], FP8, tag="rowst",
                             name="rowst")
        t16r = persist.tile([1, max(m_sh, n)], F16, tag="t16r", name="t16r")

        def sq_rows(src8, width, scol, dst_rows, nslices):
            """matmul-reduce squares of src8 (DR) -> psum rows, then
            hi/lo fp8 split (values scaled by 1/4 via scol=s/4)."""
            for s in range(nslices):
                sl = slice(s * FD, (s + 1) * FD)
                sq8 = natp.tile([P, KC, FD], FP8, tag="sq8", bufs=2,
                                name="sq8")
                nc.gpsimd.tensor_mul(sq8[:, 0, :], src8[:, 0, sl],
                                     src8[:, 0, sl])
                nc.gpsimd.tensor_mul(sq8[:, 1, :], src8[:, 1, sl],
                                     src8[:, 1, sl])
                ps = psump.tile([16, FD], F32, tag="row", bufs=1,
                                name="ps_sq")
                nc.tensor.matmul(ps, scol, sq8, start=True, stop=True,
                                 perf_mode=DR)
                # hi = fp8(ps) (ps already scaled by scol = s/4)
                nc.scalar.activation(rowst[0:1, 0, sl], ps[0:1, :], AF.Copy)
                # t16 = ps (f16), lo = t16 - hi
                nc.scalar.activation(t16r[0:1, sl], ps[0:1, :], AF.Copy)
                nc.vector.tensor_tensor(out=rowst[0:1, 1, sl],
                                        in0=t16r[0:1, sl],
                                        in1=rowst[0:1, 0, sl],
                                        op=OP.subtract)
            nc.sync.dma_start(out=dst_rows, in_=rowst[0:1, :, :width])

        # ---- A chain: load, cast, transpose (4/bank), sq_a ----
        for t4 in range(MT // AB):
            a_nat = natp.tile([P, AB, d], F32, tag="anat", bufs=2,
                              name="a_nat")
            src = a[t4 * AB * P:(t4 + 1) * AB * P, :].rearrange(
                "(t p) d -> p t d", p=P
            )
            nc.scalar.dma_start(out=a_nat, in_=src)
            for k in range(KC):
                pt = psump.tile([P, AB * P], F32, tag="tp", bufs=1,
                                name="pt_a")
                for j in range(AB):
                    nc.tensor.transpose(
                        pt[:, j * P:(j + 1) * P],
                        a_nat[:, j, k * P:(k + 1) * P], identity,
                    )
                nc.vector.tensor_copy(
                    at8[:, k, t4 * AB * P:(t4 + 1) * AB * P], pt
                )
        # sq_a rows -> c8l[0, :, :]  (use onecol8/4 = memset 0.25 too;
        # A is unscaled so scol must be 1/4 = qcol8)
        sq_rows(at8, m_sh, qcol8, c8l[0:1, :, :], m_sh // FD)

        # ---- per half: B load/cast/transpose + sqb, then main loop ----
        nbh = (NB // OH)          # b row tiles per half (32)
        for h in range(OH):
            for q in range(nbh // AB):
                t0 = h * nbh + q * AB
                b_nat = natp.tile([P, AB, d], F32, tag="bnat", bufs=6,
                                  name="b_nat")
                src = b[t0 * P:(t0 + AB) * P, :].rearrange(
                    "(t p) d -> p t d", p=P
                )
                nc.scalar.dma_start(out=b_nat, in_=src)
                for k in range(KC):
                    pt = psump.tile([P, AB * P], F32, tag="tp", bufs=1,
                                    name="pt_b")
                    for j in range(AB):
                        nc.tensor.transpose(
                            pt[:, j * P:(j + 1) * P],
                            b_nat[:, j, k * P:(k + 1) * P], identity,
                        )
                    nc.vector.tensor_scalar_mul(
                        bt8[:, k, t0 * P:(t0 + AB) * P], pt, -2.0
                    )
            # sqb rows for this half -> c8r[1, :, half]
            sq_rows(
                bt8[:, :, h * out_w:(h + 1) * out_w], out_w, q16col8,
                c8r[1:2, :, h * out_w:(h + 1) * out_w], out_w // FD,
            )
            # main loop for this half
            for m in range(MT):
                msl = slice(m * P, (m + 1) * P)
                ostage = outp.tile([P, out_w], F16, tag="ostage",
                                   name="ostage")
                for ei, (off, w) in enumerate(EV):
                    wide = psump.tile([P, 1536], F32, tag="mm", bufs=2,
                                      name="ps_mm")
                    for si in range(w // FD):
                        nsl = slice(h * out_w + off + si * FD,
                                    h * out_w + off + (si + 1) * FD)
                        dst = wide[:, si * FD:(si + 1) * FD]
                        nc.tensor.matmul(dst, at8[:, :, msl],
                                         bt8[:, :, nsl], start=True,
                                         stop=False, perf_mode=DR)
                        nc.tensor.matmul(dst, c8l[:, :, msl],
                                         c8r[:, :, nsl], start=False,
                                         stop=True, perf_mode=DR)
                    osl = ostage[:, off:off + w]
                    if (m + ei) % 2 == 0:
                        nc.vector.tensor_copy(osl, wide[:, :w])
                    else:
                        nc.scalar.activation(osl, wide[:, :w], AF.Copy)
                nc.sync.dma_start(
                    out=o[msl, h * out_w:(h + 1) * out_w], in_=ostage
                )
    nc.finalize()
    return nc


_CACHE = {}


def _get_nc():
    if "nc" not in _CACHE:
        _CACHE["nc"] = build()
    return _CACHE["nc"]


def run(mat_1, mat_2, trace=False, **kw):
    from concourse.bass_utils import run_bass_kernel_spmd

    a = np.ascontiguousarray(np.asarray(mat_1, dtype=np.float32))
    b = np.ascontiguousarray(np.asarray(mat_2, dtype=np.float32))
    assert a.shape == (M_FULL, D_FULL) and b.shape == (N_FULL, D_FULL)
    m_sh = M_FULL // N_CORES
    nc = _get_nc()
    in_maps = [
        {"a": a[c * m_sh:(c + 1) * m_sh], "b": b} for c in range(N_CORES)
    ]
    res = run_bass_kernel_spmd(
        nc, in_maps, core_ids=list(range(N_CORES)), trace=trace, **kw
    )
    out = np.concatenate(
        [np.asarray(r["out"], dtype=np.float32) for r in res.results], axis=0
    )
    return out, res


def kernel(mat_1, mat_2):
    return run(mat_1, mat_2)[0]


# revision 12
# speedup vs baseline: 1.6362x; 1.0215x over previous
"""Squared Euclidean distance matrix kernel for Trainium2 (Bass/Tile).

out[i, j] = ||mat_1[i]||^2 + ||mat_2[j]||^2 - 2 * mat_1[i] . mat_2[j]

Sharding: mat_1 rows (and output rows) split across 8 NeuronCores; mat_2
replicated.  Each core computes a (2048, 8192) tile independently.

v5 design (all-fp8-DoubleRow PE stream + fp16 output), based on HW
microbenchmarks:
  - at8 = fp8(A^T) [128, 2, 2048], bt8 = fp8(-2 B^T) [128, 2, 8192]
    (PE f32 transposes 4-at-a-time into one PSUM bank, 512-wide DVE
    scale-casts out).
  - Main tile (m, s): ONE DoubleRow fp8 matmul contracts all 256 dims
    (measured 216 ns), plus ONE zero-padded DoubleRow correction matmul
    (full K=128 partitions, only rows 0-1 nonzero; small-K matmuls and
    mixed-mode groups stall the PE stream, zero-padded same-shape DR
    does not):
      corr_l[0,c,m] = fp8(sq_a[m]/4) (hi, lo residual), corr_l[1,c,m]=4
      corr_r[0,c,n] = 4,  corr_r[1,c,n] = fp8(sq_b[n]/4) (hi, lo)
    so the pair contributes sq_a[m] + sq_b[n] exactly up to the fp8 lo
    residual (~0.5 abs).  sq_* are computed from the QUANTIZED tiles
    (squares on GPSIMD, fp8 0.25/1.0-column DoubleRow reduce on PE).
  - PSUM: two 1536-col (3-bank) main tiles double-buffered + 1 transpose
    bank + 1 row bank.  Evacuation in 1536/1024-wide chunks, split
    between DVE tensor_copy and ACT Copy, into fp16 staging; 1 MB DMA
    per (m, half).  Host upcasts fp16 -> f32.
"""

import numpy as np

import concourse.bass as bass
import concourse.mybir as mybir
from concourse import bacc
from contextlib import ExitStack
from concourse.tile import TileContext
from concourse.masks import make_identity

F32 = mybir.dt.float32
F32R = mybir.dt.float32r
F16 = mybir.dt.float16
FP8 = mybir.dt.float8e4
AX = mybir.AxisListType
OP = mybir.AluOpType
AF = mybir.ActivationFunctionType
DR = mybir.MatmulPerfMode.DoubleRow

N_CORES = 8
M_FULL, N_FULL, D_FULL = 16384, 8192, 256


def build(m_sh=M_FULL // N_CORES, n=N_FULL, d=D_FULL):
    P = 128
    FD = 512                      # psum bank width (f32)
    KC = d // P                   # 2 -> DoubleRow depth
    MT = m_sh // P                # m tiles per core (16)
    NB = n // P                   # b row tiles (64)
    AB = 4                        # row tiles per load batch
    out_w = 4096                  # out staging width == half width
    OH = n // out_w               # halves (2)
    SPH = out_w // FD             # 512-slices per half (8)
    # evacuation chunking per (m, half): 1536+1536+1024
    EV = [(0, 1536), (1536, 1536), (3072, 1024)]

    nc = bacc.Bacc()
    a = nc.dram_tensor("a", [m_sh, d], F32, kind="ExternalInput")
    b = nc.dram_tensor("b", [n, d], F32, kind="ExternalInput")
    o = nc.dram_tensor("out", [m_sh, n], F16, kind="ExternalOutput")

    with ExitStack() as ctx:
        tc = ctx.enter_context(TileContext(nc))
        singles = ctx.enter_context(tc.tile_pool(name="singles", bufs=1))
        persist = ctx.enter_context(tc.tile_pool(name="persist", bufs=1))
        natp = ctx.enter_context(tc.tile_pool(name="natp", bufs=3))
        outp = ctx.enter_context(tc.tile_pool(name="outp", bufs=2))
        psump = ctx.enter_context(tc.tile_pool(name="psump", bufs=1, space="PSUM"))

        identity = singles.tile([P, P], F32, tag="identity", name="identity")
        make_identity(nc, identity)

        # fp8 DR reduce columns [128, 2, 1]:
        #   A: sq_a/4 = 0.25 * sum(a^2)            -> 0.25
        #   B: sq_b/4 = (1/16) * sum((-2b)^2)      -> 0.0625
        # (DR weights need pair-stride %16 == 0 -> 16 columns wide, all
        # equal; the reduce result lands in psum rows 0..15, row 0 used)
        qcol8 = singles.tile([P, KC, 16], FP8, tag="qcol8", name="qcol8")
        nc.vector.memset(qcol8, 0.25)
        q16col8 = singles.tile([P, KC, 16], FP8, tag="q16col8", name="q16col8")
        nc.vector.memset(q16col8, 0.0625)

        # fp8 DR operand tiles
        bt8 = persist.tile([P, KC, n], FP8, tag="bt8", name="bt8")
        at8 = persist.tile([P, KC, m_sh], FP8, tag="at8", name="at8")
        # zero-padded DR correction operands
        c8l = persist.tile([P, KC, m_sh], FP8, tag="c8l", name="c8l")
        c8r = persist.tile([P, KC, n], FP8, tag="c8r", name="c8r")
        nc.gpsimd.memset(c8l, 0.0)
        nc.gpsimd.memset(c8r, 0.0)
        # compute engines cannot address partition offset 1; stage the 4.0
        # rows at partition 0 and DMA into place
        fours = singles.tile([1, KC, n], FP8, tag="fours", name="fours")
        nc.vector.memset(fours, 4.0)
        nc.gpsimd.dma_start(out=c8l[1:2, :, :], in_=fours[0:1, :, :m_sh])
        nc.gpsimd.dma_start(out=c8r[0:1, :, :], in_=fours[0:1, :, :])

        # row staging (partition 0), DMA'd into c8l/c8r partition rows
        rowst = persist.tile([1, 2, max(m_sh, n)], FP8, tag="rowst",
                             name="rowst")
        t16r = persist.tile([1, max(m_sh, n)], F16, tag="t16r", name="t16r")

        def sq_rows(src8, width, scol, dst_rows, nslices):
            """matmul-reduce squares of src8 (DR) -> psum rows, then
            hi/lo fp8 split (values scaled by 1/4 via scol=s/4)."""
            for s in range(nslices):
                sl = slice(s * FD, (s + 1) * FD)
                sq8 = natp.tile([P, KC, FD], FP8, tag="sq8", bufs=2,
                                name="sq8")
                nc.gpsimd.tensor_mul(sq8[:, 0, :], src8[:, 0, sl],
                                     src8[:, 0, sl])
                nc.gpsimd.tensor_mul(sq8[:, 1, :], src8[:, 1, sl],
                                     src8[:, 1, sl])
                ps = psump.tile([16, FD], F32, tag="row", bufs=1,
                                name="ps_sq")
                nc.tensor.matmul(ps, scol, sq8, start=True, stop=True,
                                 perf_mode=DR)
                # hi = fp8(ps) (ps already scaled by scol = s/4)
                nc.scalar.activation(rowst[0:1, 0, sl], ps[0:1, :], AF.Copy)
                # t16 = ps (f16), lo = t16 - hi
                nc.scalar.activation(t16r[0:1, sl], ps[0:1, :], AF.Copy)
                nc.vector.tensor_tensor(out=rowst[0:1, 1, sl],
                                        in0=t16r[0:1, sl],
                                        in1=rowst[0:1, 0, sl],
                                        op=OP.subtract)
            nc.sync.dma_start(out=dst_rows, in_=rowst[0:1, :, :width])

        # ---- A chain: load, cast, transpose (4/bank), sq_a ----
        for t4 in range(MT // AB):
            a_nat = natp.tile([P, AB, d], F32, tag="anat", bufs=2,
                              name="a_nat")
            src = a[t4 * AB * P:(t4 + 1) * AB * P, :].rearrange(
                "(t p) d -> p t d", p=P
            )
            nc.scalar.dma_start(out=a_nat, in_=src)
            for k in range(KC):
                pt = psump.tile([P, AB * P], F32, tag="tp", bufs=1,
                                name="pt_a")
                for j in range(AB):
                    nc.tensor.transpose(
                        pt[:, j * P:(j + 1) * P],
                        a_nat[:, j, k * P:(k + 1) * P], identity,
                    )
                nc.vector.tensor_copy(
                    at8[:, k, t4 * AB * P:(t4 + 1) * AB * P], pt
                )
        # sq_a rows -> c8l[0, :, :]  (use onecol8/4 = memset 0.25 too;
        # A is unscaled so scol must be 1/4 = qcol8)
        sq_rows(at8, m_sh, qcol8, c8l[0:1, :, :], m_sh // FD)

        # ---- per half: B load/cast/transpose + sqb, then main loop ----
        nbh = (NB // OH)          # b row tiles per half (32)
        for h in range(OH):
            for q in range(nbh // AB):
                t0 = h * nbh + q * AB
                b_nat = natp.tile([P, AB, d], F32, tag="bnat", bufs=6,
                                  name="b_nat")
                src = b[t0 * P:(t0 + AB) * P, :].rearrange(
                    "(t p) d -> p t d", p=P
                )
                nc.scalar.dma_start(out=b_nat, in_=src)
                for k in range(KC):
                    pt = psump.tile([P, AB * P], F32, tag="tp", bufs=1,
                                    name="pt_b")
                    for j in range(AB):
                        nc.tensor.transpose(
                            pt[:, j * P:(j + 1) * P],
                            b_nat[:, j, k * P:(k + 1) * P], identity,
                        )
                    nc.vector.tensor_scalar_mul(
                        bt8[:, k, t0 * P:(t0 + AB) * P], pt, -2.0
                    )
            # sqb rows for this half -> c8r[1, :, half]
            sq_rows(
                bt8[:, :, h * out_w:(h + 1) * out_w], out_w, q16col8,
                c8r[1:2, :, h * out_w:(h + 1) * out_w], out_w // FD,
            )
            # main loop for this half
            for m in range(MT):
                msl = slice(m * P, (m + 1) * P)
                ostage = outp.tile([P, out_w], F16, tag="ostage",
                                   name="ostage")
                for ei, (off, w) in enumerate(EV):
                    wide = psump.tile([P, 1536], F32, tag="mm", bufs=2,
                                      name="ps_mm")
                    for si in range(w // FD):
                        nsl = slice(h * out_w + off + si * FD,
                                    h * out_w + off + (si + 1) * FD)
                        dst = wide[:, si * FD:(si + 1) * FD]
                        nc.tensor.matmul(dst, at8[:, :, msl],
                                         bt8[:, :, nsl], start=True,
                                         stop=False, perf_mode=DR)
                        nc.tensor.matmul(dst, c8l[:, :, msl],
                                         c8r[:, :, nsl], start=False,
                                         stop=True, perf_mode=DR)
                    # split each evacuation across DVE and ACT so the
                    # combined rate beats the PE fill rate
                    hw_ = w // 2
                    nc.vector.tensor_copy(ostage[:, off:off + hw_],
                                          wide[:, :hw_])
                    nc.scalar.activation(ostage[:, off + hw_:off + w],
                                         wide[:, hw_:w], AF.Copy)
                nc.sync.dma_start(
                    out=o[msl, h * out_w:(h + 1) * out_w], in_=ostage
                )
    nc.finalize()
    return nc


_CACHE = {}


def _get_nc():
    if "nc" not in _CACHE:
        _CACHE["nc"] = build()
    return _CACHE["nc"]


def run(mat_1, mat_2, trace=False, **kw):
    from concourse.bass_utils import run_bass_kernel_spmd

    a = np.ascontiguousarray(np.asarray(mat_1, dtype=np.float32))
    b = np.ascontiguousarray(np.asarray(mat_2, dtype=np.float32))
    assert a.shape == (M_FULL, D_FULL) and b.shape == (N_FULL, D_FULL)
    m_sh = M_FULL // N_CORES
    nc = _get_nc()
    in_maps = [
        {"a": a[c * m_sh:(c + 1) * m_sh], "b": b} for c in range(N_CORES)
    ]
    res = run_bass_kernel_spmd(
        nc, in_maps, core_ids=list(range(N_CORES)), trace=trace, **kw
    )
    out = np.concatenate(
        [np.asarray(r["out"], dtype=np.float32) for r in res.results], axis=0
    )
    return out, res


def kernel(mat_1, mat_2):
    return run(mat_1, mat_2)[0]


# revision 13
# speedup vs baseline: 1.7299x; 1.0573x over previous
"""Squared Euclidean distance matrix kernel for Trainium2 (Bass/Tile).

out[i, j] = ||mat_1[i]||^2 + ||mat_2[j]||^2 - 2 * mat_1[i] . mat_2[j]

Sharding: mat_1 rows (and output rows) split across 8 NeuronCores; mat_2
replicated.  Each core computes a (2048, 8192) tile independently.

v5 design (all-fp8-DoubleRow PE stream + fp16 output), based on HW
microbenchmarks:
  - at8 = fp8(A^T) [128, 2, 2048], bt8 = fp8(-2 B^T) [128, 2, 8192]
    (PE f32 transposes 4-at-a-time into one PSUM bank, 512-wide DVE
    scale-casts out).
  - Main tile (m, s): ONE DoubleRow fp8 matmul contracts all 256 dims
    (measured 216 ns), plus ONE zero-padded DoubleRow correction matmul
    (full K=128 partitions, only rows 0-1 nonzero; small-K matmuls and
    mixed-mode groups stall the PE stream, zero-padded same-shape DR
    does not):
      corr_l[0,c,m] = fp8(sq_a[m]/4) (hi, lo residual), corr_l[1,c,m]=4
      corr_r[0,c,n] = 4,  corr_r[1,c,n] = fp8(sq_b[n]/4) (hi, lo)
    so the pair contributes sq_a[m] + sq_b[n] exactly up to the fp8 lo
    residual (~0.5 abs).  sq_* are computed from the QUANTIZED tiles
    (squares on GPSIMD, fp8 0.25/1.0-column DoubleRow reduce on PE).
  - PSUM: two 1536-col (3-bank) main tiles double-buffered + 1 transpose
    bank + 1 row bank.  Evacuation in 1536/1024-wide chunks, split
    between DVE tensor_copy and ACT Copy, into fp16 staging; 1 MB DMA
    per (m, half).  Host upcasts fp16 -> f32.
"""

import numpy as np

import concourse.bass as bass
import concourse.mybir as mybir
from concourse import bacc
from contextlib import ExitStack
from concourse.tile import TileContext
from concourse.masks import make_identity

F32 = mybir.dt.float32
F32R = mybir.dt.float32r
F16 = mybir.dt.float16
FP8 = mybir.dt.float8e4
AX = mybir.AxisListType
OP = mybir.AluOpType
AF = mybir.ActivationFunctionType
DR = mybir.MatmulPerfMode.DoubleRow

N_CORES = 8
M_FULL, N_FULL, D_FULL = 16384, 8192, 256


def build(m_sh=M_FULL // N_CORES, n=N_FULL, d=D_FULL):
    P = 128
    FD = 512                      # psum bank width (f32)
    KC = d // P                   # 2 -> DoubleRow depth
    MT = m_sh // P                # m tiles per core (16)
    NB = n // P                   # b row tiles (64)
    AB = 4                        # row tiles per load batch
    out_w = 4096                  # out staging width == half width
    OH = n // out_w               # halves (2)
    GW = 1024                     # psum group width (2 banks)

    nc = bacc.Bacc()
    a = nc.dram_tensor("a", [m_sh, d], F32, kind="ExternalInput")
    b = nc.dram_tensor("b", [n, d], F32, kind="ExternalInput")
    o = nc.dram_tensor("out", [m_sh, n], F16, kind="ExternalOutput")

    with ExitStack() as ctx:
        tc = ctx.enter_context(TileContext(nc))
        singles = ctx.enter_context(tc.tile_pool(name="singles", bufs=1))
        persist = ctx.enter_context(tc.tile_pool(name="persist", bufs=1))
        natp = ctx.enter_context(tc.tile_pool(name="natp", bufs=3))
        outp = ctx.enter_context(tc.tile_pool(name="outp", bufs=2))
        psump = ctx.enter_context(tc.tile_pool(name="psump", bufs=1, space="PSUM"))

        identity = singles.tile([P, P], F32, tag="identity", name="identity")
        make_identity(nc, identity)

        # fp8 DR reduce columns [128, 2, 1]:
        #   A: sq_a/4 = 0.25 * sum(a^2)            -> 0.25
        #   B: sq_b/4 = (1/16) * sum((-2b)^2)      -> 0.0625
        # (DR weights need pair-stride %16 == 0 -> 16 columns wide, all
        # equal; the reduce result lands in psum rows 0..15, row 0 used)
        qcol8 = singles.tile([P, KC, 16], FP8, tag="qcol8", name="qcol8")
        nc.vector.memset(qcol8, 0.25)
        q16col8 = singles.tile([P, KC, 16], FP8, tag="q16col8", name="q16col8")
        nc.vector.memset(q16col8, 0.0625)

        # fp8 DR operand tiles
        bt8 = persist.tile([P, KC, n], FP8, tag="bt8", name="bt8")
        at8 = persist.tile([P, KC, m_sh], FP8, tag="at8", name="at8")
        # zero-padded DR correction operands
        c8l = persist.tile([P, KC, m_sh], FP8, tag="c8l", name="c8l")
        c8r = persist.tile([P, KC, n], FP8, tag="c8r", name="c8r")
        nc.gpsimd.memset(c8l, 0.0)
        nc.gpsimd.memset(c8r, 0.0)
        # compute engines cannot address partition offset 1; stage the 4.0
        # rows at partition 0 and DMA into place
        fours = singles.tile([1, KC, n], FP8, tag="fours", name="fours")
        nc.vector.memset(fours, 4.0)
        nc.gpsimd.dma_start(out=c8l[1:2, :, :], in_=fours[0:1, :, :m_sh])
        nc.gpsimd.dma_start(out=c8r[0:1, :, :], in_=fours[0:1, :, :])

        # row staging (partition 0), DMA'd into c8l/c8r partition rows
        rowst = persist.tile([1, 2, max(m_sh, n)], FP8, tag="rowst",
                             name="rowst")
        t16r = persist.tile([1, max(m_sh, n)], F16, tag="t16r", name="t16r")

        def sq_rows(src8, width, scol, dst_rows, nslices):
            """matmul-reduce squares of src8 (DR) -> psum rows, then
            hi/lo fp8 split (values scaled by 1/4 via scol=s/4)."""
            for s in range(nslices):
                sl = slice(s * FD, (s + 1) * FD)
                sq8 = natp.tile([P, KC, FD], FP8, tag="sq8", bufs=2,
                                name="sq8")
                nc.gpsimd.tensor_mul(sq8[:, 0, :], src8[:, 0, sl],
                                     src8[:, 0, sl])
                nc.gpsimd.tensor_mul(sq8[:, 1, :], src8[:, 1, sl],
                                     src8[:, 1, sl])
                ps = psump.tile([16, FD], F32, tag="mm", bufs=4,
                                name="ps_sq")
                nc.tensor.matmul(ps, scol, sq8, start=True, stop=True,
                                 perf_mode=DR)
                # hi = fp8(ps) (ps already scaled by scol = s/4)
                nc.scalar.activation(rowst[0:1, 0, sl], ps[0:1, :], AF.Copy)
                # t16 = ps (f16), lo = t16 - hi
                nc.scalar.activation(t16r[0:1, sl], ps[0:1, :], AF.Copy)
                nc.vector.tensor_tensor(out=rowst[0:1, 1, sl],
                                        in0=t16r[0:1, sl],
                                        in1=rowst[0:1, 0, sl],
                                        op=OP.subtract)
            nc.sync.dma_start(out=dst_rows, in_=rowst[0:1, :, :width])

        # ---- A chain: load, transpose 8-at-a-time into a 1024-col psum
        # gen, 1024-wide fp8 cast out ----
        for t8 in range(MT // (2 * AB)):
            bats = []
            for i in range(2):
                a_nat = natp.tile([P, AB, d], F32, tag="anat", bufs=2,
                                  name="a_nat")
                r0 = (t8 * 2 + i) * AB * P
                src = a[r0:r0 + AB * P, :].rearrange("(t p) d -> p t d", p=P)
                nc.scalar.dma_start(out=a_nat, in_=src)
                bats.append(a_nat)
            for k in range(KC):
                pt = psump.tile([P, 2 * AB * P], F32, tag="mm", bufs=4,
                                name="pt_a")
                for jj in range(2 * AB):
                    nc.tensor.transpose(
                        pt[:, jj * P:(jj + 1) * P],
                        bats[jj // AB][:, jj % AB, k * P:(k + 1) * P],
                        identity,
                    )
                nc.vector.tensor_copy(
                    at8[:, k, t8 * 2 * AB * P:(t8 + 1) * 2 * AB * P], pt
                )
        # sq_a rows -> c8l[0, :, :]  (use onecol8/4 = memset 0.25 too;
        # A is unscaled so scol must be 1/4 = qcol8)
        sq_rows(at8, m_sh, qcol8, c8l[0:1, :, :], m_sh // FD)

        # ---- per half: B load/cast/transpose + sqb, then main loop ----
        nbh = (NB // OH)          # b row tiles per half (32)
        for h in range(OH):
            for q2 in range(nbh // (2 * AB)):
                t0 = h * nbh + q2 * 2 * AB
                bats = []
                for i in range(2):
                    b_nat = natp.tile([P, AB, d], F32, tag="bnat", bufs=6,
                                      name="b_nat")
                    r0 = (t0 + i * AB) * P
                    src = b[r0:r0 + AB * P, :].rearrange(
                        "(t p) d -> p t d", p=P
                    )
                    nc.scalar.dma_start(out=b_nat, in_=src)
                    bats.append(b_nat)
                for k in range(KC):
                    pt = psump.tile([P, 2 * AB * P], F32, tag="mm", bufs=4,
                                    name="pt_b")
                    for jj in range(2 * AB):
                        nc.tensor.transpose(
                            pt[:, jj * P:(jj + 1) * P],
                            bats[jj // AB][:, jj % AB, k * P:(k + 1) * P],
                            identity,
                        )
                    nc.vector.tensor_scalar_mul(
                        bt8[:, k, t0 * P:(t0 + 2 * AB) * P], pt, -2.0
                    )
            # sqb rows for this half -> c8r[1, :, half]
            sq_rows(
                bt8[:, :, h * out_w:(h + 1) * out_w], out_w, q16col8,
                c8r[1:2, :, h * out_w:(h + 1) * out_w], out_w // FD,
            )
            # main loop for this half
            for m in range(MT):
                msl = slice(m * P, (m + 1) * P)
                ostage = outp.tile([P, out_w], F16, tag="ostage",
                                   name="ostage")
                for g in range(out_w // GW):
                    off = g * GW
                    wide = psump.tile([P, GW], F32, tag="mm", bufs=4,
                                      name="ps_mm")
                    for si in range(GW // FD):
                        nsl = slice(h * out_w + off + si * FD,
                                    h * out_w + off + (si + 1) * FD)
                        dst = wide[:, si * FD:(si + 1) * FD]
                        nc.tensor.matmul(dst, at8[:, :, msl],
                                         bt8[:, :, nsl], start=True,
                                         stop=False, perf_mode=DR)
                        nc.tensor.matmul(dst, c8l[:, :, msl],
                                         c8r[:, :, nsl], start=False,
                                         stop=True, perf_mode=DR)
                    # split each evacuation across DVE and ACT so the
                    # combined rate beats the PE fill rate
                    nc.vector.tensor_copy(ostage[:, off:off + GW // 2],
                                          wide[:, :GW // 2])
                    nc.scalar.activation(ostage[:, off + GW // 2:off + GW],
                                         wide[:, GW // 2:], AF.Copy)
                nc.sync.dma_start(
                    out=o[msl, h * out_w:(h + 1) * out_w], in_=ostage
                )
    nc.finalize()
    return nc


_CACHE = {}


def _get_nc():
    if "nc" not in _CACHE:
        _CACHE["nc"] = build()
    return _CACHE["nc"]


def run(mat_1, mat_2, trace=False, **kw):
    from concourse.bass_utils import run_bass_kernel_spmd

    a = np.ascontiguousarray(np.asarray(mat_1, dtype=np.float32))
    b = np.ascontiguousarray(np.asarray(mat_2, dtype=np.float32))
    assert a.shape == (M_FULL, D_FULL) and b.shape == (N_FULL, D_FULL)
    m_sh = M_FULL // N_CORES
    nc = _get_nc()
    in_maps = [
        {"a": a[c * m_sh:(c + 1) * m_sh], "b": b} for c in range(N_CORES)
    ]
    res = run_bass_kernel_spmd(
        nc, in_maps, core_ids=list(range(N_CORES)), trace=trace, **kw
    )
    out = np.concatenate(
        [np.asarray(r["out"], dtype=np.float32) for r in res.results], axis=0
    )
    return out, res


def kernel(mat_1, mat_2):
    return run(mat_1, mat_2)[0]


# revision 16
# speedup vs baseline: 2.1601x; 1.2487x over previous
"""Squared Euclidean distance matrix kernel for Trainium2 (Bass/Tile).

out[i, j] = ||mat_1[i]||^2 + ||mat_2[j]||^2 - 2 * mat_1[i] . mat_2[j]

Sharding: mat_1 rows (and output rows) split across 8 NeuronCores; mat_2
replicated.  Each core computes a (2048, 8192) tile independently.

v5 design (all-fp8-DoubleRow PE stream + fp16 output), based on HW
microbenchmarks:
  - at8 = fp8(A^T) [128, 2, 2048], bt8 = fp8(-2 B^T) [128, 2, 8192]
    (PE f32 transposes 4-at-a-time into one PSUM bank, 512-wide DVE
    scale-casts out).
  - Main tile (m, s): ONE DoubleRow fp8 matmul contracts all 256 dims
    (measured 216 ns), plus ONE zero-padded DoubleRow correction matmul
    (full K=128 partitions, only rows 0-1 nonzero; small-K matmuls and
    mixed-mode groups stall the PE stream, zero-padded same-shape DR
    does not):
      corr_l[0,c,m] = fp8(sq_a[m]/4) (hi, lo residual), corr_l[1,c,m]=4
      corr_r[0,c,n] = 4,  corr_r[1,c,n] = fp8(sq_b[n]/4) (hi, lo)
    so the pair contributes sq_a[m] + sq_b[n] exactly up to the fp8 lo
    residual (~0.5 abs).  sq_* are computed from the QUANTIZED tiles
    (squares on GPSIMD, fp8 0.25/1.0-column DoubleRow reduce on PE).
  - PSUM: two 1536-col (3-bank) main tiles double-buffered + 1 transpose
    bank + 1 row bank.  Evacuation in 1536/1024-wide chunks, split
    between DVE tensor_copy and ACT Copy, into fp16 staging; 1 MB DMA
    per (m, half).  Host upcasts fp16 -> f32.
"""

import numpy as np

import concourse.bass as bass
import concourse.mybir as mybir
from concourse import bacc
from contextlib import ExitStack
from concourse.tile import TileContext
from concourse.masks import make_identity

F32 = mybir.dt.float32
F32R = mybir.dt.float32r
F16 = mybir.dt.float16
FP8 = mybir.dt.float8e4
AX = mybir.AxisListType
OP = mybir.AluOpType
AF = mybir.ActivationFunctionType
DR = mybir.MatmulPerfMode.DoubleRow

N_CORES = 8
M_FULL, N_FULL, D_FULL = 16384, 8192, 256


def build(m_sh=M_FULL // N_CORES, n=N_FULL, d=D_FULL):
    P = 128
    FD = 512                      # psum bank width (f32)
    KC = d // P                   # 2 -> DoubleRow depth
    MT = m_sh // P                # m tiles per core (16)
    NB = n // P                   # b row tiles (64)
    AB = 4                        # row tiles per load batch
    out_w = 4096                  # out staging width == half width
    OH = n // out_w               # halves (2)
    GW = 1024                     # psum group width (2 banks)

    nc = bacc.Bacc()
    a = nc.dram_tensor("a", [m_sh, d], F32, kind="ExternalInput")
    b = nc.dram_tensor("b", [n, d], F32, kind="ExternalInput")
    o = nc.dram_tensor("out", [m_sh, n], F16, kind="ExternalOutput")

    with ExitStack() as ctx:
        tc = ctx.enter_context(TileContext(nc))
        singles = ctx.enter_context(tc.tile_pool(name="singles", bufs=1))
        persist = ctx.enter_context(tc.tile_pool(name="persist", bufs=1))
        natp = ctx.enter_context(tc.tile_pool(name="natp", bufs=3))
        outp = ctx.enter_context(tc.tile_pool(name="outp", bufs=3))
        psump = ctx.enter_context(tc.tile_pool(name="psump", bufs=1, space="PSUM"))

        identity = singles.tile([P, P], F32, tag="identity", name="identity")
        make_identity(nc, identity)

        # fp8 DR reduce columns [128, 2, 1]:
        #   A: sq_a/4 = 0.25 * sum(a^2)            -> 0.25
        #   B: sq_b/4 = (1/16) * sum((-2b)^2)      -> 0.0625
        # (DR weights need pair-stride %16 == 0 -> 16 columns wide, all
        # equal; the reduce result lands in psum rows 0..15, row 0 used)
        qcol8 = singles.tile([P, KC, 16], FP8, tag="qcol8", name="qcol8")
        nc.vector.memset(qcol8, 0.25)
        q16col8 = singles.tile([P, KC, 16], FP8, tag="q16col8", name="q16col8")
        nc.vector.memset(q16col8, 0.0625)

        # fp8 DR operand tiles
        bt8 = persist.tile([P, KC, n], FP8, tag="bt8", name="bt8")
        at8 = persist.tile([P, KC, m_sh], FP8, tag="at8", name="at8")
        # zero-padded DR correction operands
        c8l = persist.tile([P, KC, m_sh], FP8, tag="c8l", name="c8l")
        c8r = persist.tile([P, KC, n], FP8, tag="c8r", name="c8r")
        nc.gpsimd.memset(c8l, 0.0)
        nc.gpsimd.memset(c8r, 0.0)
        # compute engines cannot address partition offset 1; stage the 4.0
        # rows at partition 0 and DMA into place
        fours = singles.tile([1, KC, n], FP8, tag="fours", name="fours")
        nc.vector.memset(fours, 4.0)
        nc.gpsimd.dma_start(out=c8l[1:2, :, :], in_=fours[0:1, :, :m_sh])
        nc.gpsimd.dma_start(out=c8r[0:1, :, :], in_=fours[0:1, :, :])

        # row staging (partition 0), DMA'd into c8l/c8r partition rows
        rowst = persist.tile([1, 2, max(m_sh, n)], FP8, tag="rowst",
                             name="rowst")
        t16r = persist.tile([1, max(m_sh, n)], F16, tag="t16r", name="t16r")

        def sq_rows(src8, width, scol, dst_rows, nslices):
            """matmul-reduce squares of src8 (DR) -> psum rows, then
            hi/lo fp8 split (values scaled by 1/4 via scol=s/4)."""
            for s in range(nslices):
                sl = slice(s * FD, (s + 1) * FD)
                sq8 = natp.tile([P, KC, FD], FP8, tag="sq8", bufs=2,
                                name="sq8")
                nc.gpsimd.tensor_mul(sq8[:, 0, :], src8[:, 0, sl],
                                     src8[:, 0, sl])
                nc.gpsimd.tensor_mul(sq8[:, 1, :], src8[:, 1, sl],
                                     src8[:, 1, sl])
                ps = psump.tile([16, FD], F32, tag="mm", bufs=4,
                                name="ps_sq")
                nc.tensor.matmul(ps, scol, sq8, start=True, stop=True,
                                 perf_mode=DR)
                # hi = fp8(ps) (ps already scaled by scol = s/4)
                nc.scalar.activation(rowst[0:1, 0, sl], ps[0:1, :], AF.Copy)
                # t16 = ps (f16), lo = t16 - hi
                nc.scalar.activation(t16r[0:1, sl], ps[0:1, :], AF.Copy)
                nc.vector.tensor_tensor(out=rowst[0:1, 1, sl],
                                        in0=t16r[0:1, sl],
                                        in1=rowst[0:1, 0, sl],
                                        op=OP.subtract)
            nc.sync.dma_start(out=dst_rows, in_=rowst[0:1, :, :width])

        # ---- A chain: ALL loads issued first (the DMA queue streams
        # while the PE transposes trail the data), then transpose
        # 8-at-a-time into a 1024-col psum gen, 1024-wide fp8 cast out ----
        a_bats = []
        for i in range(MT // AB):
            a_nat = natp.tile([P, AB, d], F32, tag="anat", bufs=MT // AB,
                              name="a_nat")
            r0 = i * AB * P
            src = a[r0:r0 + AB * P, :].rearrange("(t p) d -> p t d", p=P)
            nc.sync.dma_start(out=a_nat, in_=src)
            a_bats.append(a_nat)
        for t8 in range(MT // (2 * AB)):
            bats = a_bats[2 * t8:2 * t8 + 2]
            for k in range(KC):
                pt = psump.tile([P, 2 * AB * P], F32, tag="mm", bufs=4,
                                name="pt_a")
                for jj in range(2 * AB):
                    nc.tensor.transpose(
                        pt[:, jj * P:(jj + 1) * P],
                        bats[jj // AB][:, jj % AB, k * P:(k + 1) * P],
                        identity,
                    )
                nc.vector.tensor_copy(
                    at8[:, k, t8 * 2 * AB * P:(t8 + 1) * 2 * AB * P], pt
                )
        # sq_a rows -> c8l[0, :, :]  (use onecol8/4 = memset 0.25 too;
        # A is unscaled so scol must be 1/4 = qcol8)
        sq_rows(at8, m_sh, qcol8, c8l[0:1, :, :], m_sh // FD)

        # ---- per half: B prep emitted as chunks; the main loop of the
        # PREVIOUS half interleaves the next half's prep chunks so the PE
        # never hits a prep bubble at the half boundary ----
        nbh = (NB // OH)          # b row tiles per half (32)

        def b_prep_chunks(h):
            """Yield prep work for half h in small chunks.  All loads for
            the half are issued in the first chunk (8 bufs alive)."""
            h_bats = []
            for i in range(nbh // AB):
                b_nat = natp.tile([P, AB, d], F32, tag="bnat",
                                  bufs=nbh // AB, name="b_nat")
                r0 = (h * nbh + i * AB) * P
                src = b[r0:r0 + AB * P, :].rearrange("(t p) d -> p t d", p=P)
                nc.sync.dma_start(out=b_nat, in_=src)
                h_bats.append(b_nat)
            for q2 in range(nbh // (2 * AB)):
                t0 = h * nbh + q2 * 2 * AB
                bats = h_bats[2 * q2:2 * q2 + 2]
                for k in range(KC):
                    pt = psump.tile([P, 2 * AB * P], F32, tag="mm", bufs=4,
                                    name="pt_b")
                    for jj in range(2 * AB):
                        nc.tensor.transpose(
                            pt[:, jj * P:(jj + 1) * P],
                            bats[jj // AB][:, jj % AB, k * P:(k + 1) * P],
                            identity,
                        )
                    nc.vector.tensor_scalar_mul(
                        bt8[:, k, t0 * P:(t0 + 2 * AB) * P], pt, -2.0
                    )
                yield
            # sqb rows for this half -> c8r[1, :, half] (one slice per chunk)
            for s in range(out_w // FD):
                sl = slice(s * FD, (s + 1) * FD)
                asl = slice(h * out_w + s * FD, h * out_w + (s + 1) * FD)
                sq8 = natp.tile([P, KC, FD], FP8, tag="sq8", bufs=2,
                                name="sq8")
                nc.gpsimd.tensor_mul(sq8[:, 0, :], bt8[:, 0, asl],
                                     bt8[:, 0, asl])
                nc.gpsimd.tensor_mul(sq8[:, 1, :], bt8[:, 1, asl],
                                     bt8[:, 1, asl])
                ps = psump.tile([16, FD], F32, tag="mm", bufs=4,
                                name="ps_sq")
                nc.tensor.matmul(ps, q16col8, sq8, start=True, stop=True,
                                 perf_mode=DR)
                nc.scalar.activation(rowst[0:1, 0, sl], ps[0:1, :], AF.Copy)
                nc.scalar.activation(t16r[0:1, sl], ps[0:1, :], AF.Copy)
                nc.vector.tensor_tensor(out=rowst[0:1, 1, sl],
                                        in0=t16r[0:1, sl],
                                        in1=rowst[0:1, 0, sl],
                                        op=OP.subtract)
                yield
            nc.sync.dma_start(
                out=c8r[1:2, :, h * out_w:(h + 1) * out_w],
                in_=rowst[0:1, :, :out_w],
            )
            yield

        def main_half(h, prep):
            """Main loop for half h, pulling prep chunks for half h+1."""
            for m in range(MT):
                if prep is not None and m >= 2:
                    for _ in range(2):
                        next(prep, None)
                msl = slice(m * P, (m + 1) * P)
                ostage = outp.tile([P, out_w], F16, tag="ostage",
                                   name="ostage")
                for g in range(out_w // GW):
                    off = g * GW
                    wide = psump.tile([P, GW], F32, tag="mm", bufs=4,
                                      name="ps_mm")
                    for si in range(GW // FD):
                        nsl = slice(h * out_w + off + si * FD,
                                    h * out_w + off + (si + 1) * FD)
                        dst = wide[:, si * FD:(si + 1) * FD]
                        nc.tensor.matmul(dst, at8[:, :, msl],
                                         bt8[:, :, nsl], start=True,
                                         stop=False, perf_mode=DR)
                        nc.tensor.matmul(dst, c8l[:, :, msl],
                                         c8r[:, :, nsl], start=False,
                                         stop=True, perf_mode=DR)
                    # split each evacuation across DVE and ACT so the
                    # combined rate beats the PE fill rate
                    nc.vector.tensor_copy(ostage[:, off:off + GW // 2],
                                          wide[:, :GW // 2])
                    nc.scalar.activation(ostage[:, off + GW // 2:off + GW],
                                         wide[:, GW // 2:], AF.Copy)
                nc.sync.dma_start(
                    out=o[msl, h * out_w:(h + 1) * out_w], in_=ostage
                )
            if prep is not None:
                for _ in prep:
                    pass

        # half 0 prep runs up front; half 1 prep interleaves with main(0)
        for _ in b_prep_chunks(0):
            pass
        main_half(0, b_prep_chunks(1))
        main_half(1, None)
    nc.finalize()
    return nc


_CACHE = {}


def _get_nc():
    if "nc" not in _CACHE:
        _CACHE["nc"] = build()
    return _CACHE["nc"]


def run(mat_1, mat_2, trace=False, **kw):
    from concourse.bass_utils import run_bass_kernel_spmd

    a = np.ascontiguousarray(np.asarray(mat_1, dtype=np.float32))
    b = np.ascontiguousarray(np.asarray(mat_2, dtype=np.float32))
    assert a.shape == (M_FULL, D_FULL) and b.shape == (N_FULL, D_FULL)
    m_sh = M_FULL // N_CORES
    nc = _get_nc()
    in_maps = [
        {"a": a[c * m_sh:(c + 1) * m_sh], "b": b} for c in range(N_CORES)
    ]
    res = run_bass_kernel_spmd(
        nc, in_maps, core_ids=list(range(N_CORES)), trace=trace, **kw
    )
    out = np.concatenate(
        [np.asarray(r["out"], dtype=np.float32) for r in res.results], axis=0
    )
    return out, res


def kernel(mat_1, mat_2):
    return run(mat_1, mat_2)[0]
